# revision 14
# baseline (speedup 1.0000x reference)
"""Trainium2 Bass kernel for DeformationNetworkGraphConvolutionalFullRes.

Full (unsharded) inputs in, full output out. Data-parallel over the 4 meshes:
core m processes mesh m. Inside each core:

  - vert_align sampling as (S @ F) @ W == S @ (F @ W): per feature map,
    F[C,HW] @ Wslice[C,128] -> G[HW,128] (bf16), then the sparse bilinear
    operator S applied as dense [128px, 512vert] bf16 blocks on the
    TensorEngine, accumulating in PSUM. Vertices pre-sorted by image cell.
  - Each GraphConv layer routes its 61440 directed-edge messages through TWO
    independent engines in parallel:
      * DMA half: h1 rows written to HBM, messages pulled with dma_gather
        in dst-sorted order (DMA engines).
      * ap half: h1 kept as f32 columns in SBUF (three [128, 3584] windows);
        gpsimd.ap_gather selects message columns (Pool engine), PE
        transposes them to row form.
    Both halves are scatter-added per dst tile with one-hot matmuls
    (ap one-hots built once in fp8 and kept SBUF-resident; DMA-half
    one-hots rebuilt per group on DVE), accumulating in PSUM on top of
    h0 = W0^T x (+ rank-1 image-encoding term at layer 0); ReLU writes
    the bf16 column-form activations in place.
"""

import ml_dtypes
import numpy as np
from contextlib import ExitStack

import concourse.bass as bass
import concourse.tile as tile
from concourse import bacc, mybir
from concourse.bass_utils import run_bass_kernel_spmd

# ---------------- problem constants (hardcoded per spec) ----------------
B = 4
V = 10242
E_PER = 30720
HID = 128
MAPS = [(256, 56), (512, 28), (1024, 14), (2048, 7)]  # (C, H==W)
CH_OFF = [0, 256, 768, 1792, 3840]

VP = 10752            # padded vertex count: 84 tiles of 128
NT = VP // 128        # 84 vertex tiles
NVCH = VP // 512      # 21 vertex chunks (sampling)
NW = 3                # ap-gather source windows
WSZ = VP // NW        # 3584 columns per window
VW = V // NW          # 3414 real verts per window
GT = 8                # dst tiles per scatter group
NGRP = (NT + GT - 1) // GT  # 11 groups (last has 4 tiles)
CALL_SUBS = 32        # max ap-gather call size (subchunks of 128)
AP_FRAC = 0.50        # target fraction of edges through the ap path
TR_F32R = False       # f32r rejected by BIR verifier (needs rounded input)

F32 = mybir.dt.float32
F32R = mybir.dt.float32r
BF16 = mybir.dt.bfloat16
FP8 = mybir.dt.float8e4
I16 = mybir.dt.int16
AF = mybir.ActivationFunctionType


def _corners(grid, W):
    """grid [V,2] in [-1,1] -> list of (pix_idx int64, weight f32) per corner."""
    x = (grid[:, 0] + 1.0) * 0.5 * (W - 1)
    y = (grid[:, 1] + 1.0) * 0.5 * (W - 1)
    x0f, y0f = np.floor(x), np.floor(y)
    wx1, wy1 = (x - x0f).astype(np.float32), (y - y0f).astype(np.float32)
    wx0, wy0 = 1.0 - wx1, 1.0 - wy1
    x0 = np.clip(x0f, 0, W - 1).astype(np.int64)
    x1 = np.clip(x0f + 1, 0, W - 1).astype(np.int64)
    y0 = np.clip(y0f, 0, W - 1).astype(np.int64)
    y1 = np.clip(y0f + 1, 0, W - 1).astype(np.int64)
    return [
        (y0 * W + x0, wy0 * wx0),
        (y0 * W + x1, wy0 * wx1),
        (y1 * W + x0, wy1 * wx0),
        (y1 * W + x1, wy1 * wx1),
    ]


def _wrap16(idx):
    """int array [n] (n % 16 == 0) -> [128, n/16] wrapped+replicated for the
    8 Q7 cores (idx i at (i%16, i//16))."""
    return np.tile(idx.reshape(-1, 16).T, (8, 1)).astype(np.int16)


def _prep(inputs):
    """Host-side restructuring. Returns (cfg, per_core_aux_list, post)."""
    feats = [inputs["feat1"], inputs["feat2"], inputs["feat3"], inputs["feat4"]]
    av = np.asarray(inputs["aligned_verts"], np.float32)
    verts = np.asarray(inputs["verts_packed"], np.float32)
    enc = np.asarray(inputs["image_enc"], np.float32)
    edges = np.asarray(inputs["edges"], np.int64)

    for bn in ["bottleneck_b", "g0_b0", "g0_b1", "off_b"]:
        assert not np.any(np.asarray(inputs[bn])), f"{bn} nonzero: unsupported"
    assert not np.any(np.asarray(inputs["gb0"])) and not np.any(
        np.asarray(inputs["gb1"])
    ), "gb nonzero: unsupported"

    # per-mesh vertex sort (by finest-map cell); cell-sorted order split into
    # NW windows of VW real verts; window w occupies slots [w*WSZ, w*WSZ+VW),
    # the rest of each window is zero padding.
    sigmas, slot_of = [], []
    corners_all = []
    for m in range(B):
        grid = av[m, :, :2]
        cs = _corners(grid, MAPS[0][1])
        key = cs[0][0]
        sigma = np.argsort(key, kind="stable")
        slot = np.full(V, -1, np.int64)
        for w in range(NW):
            slot[sigma[w * VW:(w + 1) * VW]] = w * WSZ + np.arange(VW)
        sigmas.append(sigma)
        slot_of.append(slot)
        corners_all.append([_corners(grid, Wm) for (_, Wm) in MAPS])

    vert_at = []
    for m in range(B):
        va = np.full(VP, -1, np.int64)
        va[slot_of[m][np.arange(V)]] = np.arange(V)
        vert_at.append(va)

    # sampling schedule
    ntile_map = [(Wm * Wm + 127) // 128 for (_, Wm) in MAPS]
    g_off = np.cumsum([0] + ntile_map)
    sched = []
    for mi in range(4):
        per_c = []
        for c in range(NVCH):
            lo, hi = c * 512, (c + 1) * 512
            tiles = set()
            for m in range(B):
                vs = vert_at[m][lo:hi]
                vs = vs[vs >= 0]
                if len(vs):
                    for (pix, _w) in corners_all[m][mi]:
                        tiles.update(np.unique(pix[vs] // 128).tolist())
            per_c.append(sorted(tiles) if tiles else [0])
        np_m = max(len(t) for t in per_c)
        per_c = [t + [t[0]] * (np_m - len(t)) for t in per_c]
        sched.append(per_c)
    np_list = [len(sched[mi][0]) for mi in range(4)]
    npair = sum(np_list) * NVCH

    # graph structure ------------------------------------------------------
    # directed edges in slot space, sorted by (dst tile, src window)
    per_mesh_edges = []
    cnt_tw = np.zeros((B, NT, NW), np.int64)
    for m in range(B):
        e = edges[m * E_PER:(m + 1) * E_PER] - m * V
        a = slot_of[m][e[:, 0]]
        b_ = slot_of[m][e[:, 1]]
        dst = np.concatenate([a, b_])
        src = np.concatenate([b_, a])
        win = src // WSZ
        order = np.lexsort((src, win, dst // 128))
        dst, src, win = dst[order], src[order], win[order]
        per_mesh_edges.append((dst, src, win))
        tl = dst // 128
        for t in range(NT):
            sel = tl == t
            for w in range(NW):
                cnt_tw[m, t, w] = np.sum(sel & (win == w))

    # ap routing: n_ap[t][w] full subchunks through the ap path (shared)
    min_cnt = cnt_tw.min(axis=0)  # [NT, NW]
    n_ap = np.minimum(min_cnt // 128, 2).astype(np.int64)
    target_slots = int(AP_FRAC * 2 * E_PER)
    cur = int(n_ap.sum() * 128)
    order2 = np.argsort(min_cnt.reshape(-1) % 128)
    for idx in order2:
        if cur <= target_slots:
            break
        t, w = divmod(int(idx), NW)
        if n_ap[t, w] > 0:
            n_ap[t, w] -= 1
            cur -= 128
    # DMA-half subchunk counts (shared)
    rem = cnt_tw - n_ap[None] * 128
    rem_t = rem.sum(axis=2)
    nsub_dma = np.maximum(0, -(-rem_t.max(axis=0) // 128))
    sub_off = np.concatenate([[0], np.cumsum(nsub_dma)]).astype(int)
    S_dma = int(sub_off[-1])

    # ap stream layout per window: subchunk ranges per tile
    ap_off = np.zeros((NT + 1, NW), np.int64)
    for w in range(NW):
        ap_off[1:, w] = np.cumsum(n_ap[:, w])
    S_ap = [int(ap_off[NT, w]) for w in range(NW)]
    S_ap_tot = sum(S_ap)

    # ap gather call partition per stream: whole-GROUP tile ranges (so that
    # successive trmsg-buffer tenants have strictly disjoint group ranges),
    # each call <= CALL_SUBS subchunks.
    calls = []  # per w: list of (sub0, sub1, t0, t1, g_start, g_end)
    for w in range(NW):
        cl = []
        g0 = 0
        while g0 < NGRP:
            g1 = g0
            while (g1 < NGRP
                   and ap_off[min((g1 + 1) * GT, NT), w]
                   - ap_off[g0 * GT, w] <= CALL_SUBS):
                g1 += 1
            assert g1 > g0, f"group {g0} stream {w} exceeds CALL_SUBS"
            t0, t1 = g0 * GT, min(g1 * GT, NT)
            s0, s1 = int(ap_off[t0, w]), int(ap_off[t1, w])
            if s1 > s0:
                cl.append((s0, s1, t0, t1, g0, g1 - 1))
            g0 = g1
        calls.append(cl)

    # trmsg rotation safety (shared pool of 3 bufs, merged emission order):
    # emission at group g_start must come after all consumers of the buffer's
    # previous tenant (g_end of the call 3 positions earlier in merged order).
    merged = []
    for w in range(NW):
        for k, c in enumerate(calls[w]):
            merged.append((c[4], c[5], w, k))
    merged.sort()
    for i in range(3, len(merged)):
        assert merged[i][0] > merged[i - 3][1], (
            f"trmsg rotation hazard: {merged[i]} vs {merged[i-3]}")

    S_tot = S_dma + S_ap_tot

    cfg = {"sched": sched, "np_list": np_list, "g_off": g_off.tolist(),
           "ntile_map": ntile_map, "nsub_dma": nsub_dma.tolist(),
           "sub_off": sub_off.tolist(), "S_dma": S_dma,
           "n_ap": n_ap.tolist(), "ap_off": ap_off.tolist(),
           "S_ap": S_ap, "calls": calls, "S_tot": S_tot, "npair": npair}

    # ---------------- per-core tables ----------------
    per_core = []
    for m in range(B):
        dst, src, win = per_mesh_edges[m]
        ap_idx = [np.zeros(max(16, S_ap[w] * 128), np.int64) for w in range(NW)]
        ap_dl = [np.zeros(S_ap[w] * 128, np.int64) for w in range(NW)]
        src_slots = np.zeros(max(16, S_dma * 128), np.int64)
        dl_slots = np.full(S_dma * 128, -1, np.int64)
        pos = 0
        for t in range(NT):
            dma_d, dma_s = [], []
            for w in range(NW):
                c = int(cnt_tw[m, t, w])
                d_, s_ = dst[pos:pos + c], src[pos:pos + c]
                na = int(n_ap[t, w]) * 128
                o = int(ap_off[t, w]) * 128
                ap_idx[w][o:o + na] = s_[:na] - w * WSZ
                ap_dl[w][o:o + na] = d_[:na] - t * 128
                dma_d.append(d_[na:])
                dma_s.append(s_[na:])
                pos += c
            d_ = np.concatenate(dma_d)
            s_ = np.concatenate(dma_s)
            so = sub_off[t] * 128
            src_slots[so:so + len(s_)] = s_
            dl_slots[so:so + len(d_)] = d_ - t * 128
        assert pos == 2 * E_PER

        dl_all = np.concatenate([dl_slots] + ap_dl)
        dl_tab = dl_all.reshape(S_tot, 128).T.copy().astype(ml_dtypes.bfloat16)

        srcw = _wrap16(src_slots)
        apw = [_wrap16(ap_idx[w]) for w in range(NW)]

        # sampling blocks ---------------------------------------------------
        npc = sum(np_list)
        wsc = np.zeros((npair, 128, 512), np.float32)
        pi = 0
        for c in range(NVCH):
            lo = c * 512
            vs_all = vert_at[m][lo:lo + 512]
            jj = np.nonzero(vs_all >= 0)[0]
            for mi in range(4):
                seen = set()
                for t in sched[mi][c]:
                    blk = wsc[pi]
                    if t not in seen and len(jj):
                        seen.add(t)
                        for (pix, w_) in corners_all[m][mi]:
                            px = pix[vs_all[jj]]
                            sel = (px >= t * 128) & (px < (t + 1) * 128)
                            jj2 = jj[sel]
                            np.add.at(blk, (pix[vs_all[jj2]] - t * 128, jj2),
                                      w_[vs_all[jj2]])
                    pi += 1
        assert pi == npair

        vt = np.zeros((3, VP), np.float32)
        vslots = slot_of[m][np.arange(V)]
        vt[:, vslots] = verts[m * V:(m + 1) * V].T

        bf = ml_dtypes.bfloat16
        aux = {
            "f1": feats[0][m].reshape(256, -1).astype(bf),
            "f2": feats[1][m].reshape(512, -1).astype(bf),
            "f3": feats[2][m].reshape(1024, -1).astype(bf),
            "f4": feats[3][m].reshape(2048, -1).astype(bf),
            "bw": np.asarray(inputs["bottleneck_w"], np.float32).astype(bf),
            "wsc": wsc.reshape(npair * 128, 512).astype(bf),
            "srcw": np.ascontiguousarray(srcw),
            "apw0": np.ascontiguousarray(apw[0]),
            "apw1": np.ascontiguousarray(apw[1]),
            "apw2": np.ascontiguousarray(apw[2]),
            "dstloc": np.ascontiguousarray(dl_tab),
            "iota": np.tile(np.arange(128, dtype=bf), (128, 1)),
            "ident": np.eye(128, dtype=np.float32),
            "vertsT": vt,
            "encc": enc[m].reshape(2, 128).T.copy(),
            "g0w0m": np.asarray(inputs["g0_w0"][:128], np.float32).astype(bf),
            "g0w1m": np.asarray(inputs["g0_w1"][:128], np.float32).astype(bf),
            "g0w0v": np.asarray(inputs["g0_w0"][128:131], np.float32),
            "g0w1v": np.asarray(inputs["g0_w1"][128:131], np.float32),
            "g0w0e": np.ascontiguousarray(
                np.asarray(inputs["g0_w0"][131:387], np.float32)),
            "g0w1e": np.ascontiguousarray(
                np.asarray(inputs["g0_w1"][131:387], np.float32)),
            "gw0": np.ascontiguousarray(
                np.asarray(inputs["gw0"], np.float32).transpose(1, 0, 2)
                .reshape(128, 7 * 128)).astype(bf),
            "gw1": np.ascontiguousarray(
                np.asarray(inputs["gw1"], np.float32).transpose(1, 0, 2)
                .reshape(128, 7 * 128)).astype(bf),
            "offw": np.asarray(inputs["off_w"], np.float32).astype(bf),
        }
        per_core.append(aux)

    post = {"slot_of": slot_of}
    return cfg, per_core, post


def _build(cfg, shapes, nlayers=8, repeat=1):
    """Build the SPMD Bass program (same instruction stream for all cores)."""
    nc = bacc.Bacc("TRN2", target_bir_lowering=False, debug=False, num_devices=B)
    ap = {}
    for name, arr in shapes.items():
        ap[name] = nc.dram_tensor(
            name, list(arr.shape), mybir.dt.from_np(arr.dtype),
            kind="ExternalInput").ap()
    out = nc.dram_tensor("out", [VP, 3], F32, kind="ExternalOutput").ap()
    h1d2 = [nc.dram_tensor("h1da", [VP, HID], BF16).ap(),
            nc.dram_tensor("h1db", [VP, HID], BF16).ap()]

    sched = cfg["sched"]
    np_list = cfg["np_list"]
    g_off = cfg["g_off"]
    ntile_map = cfg["ntile_map"]
    NGT_ = g_off[4]
    nsub_dma = cfg["nsub_dma"]
    sub_off = cfg["sub_off"]
    S_dma = cfg["S_dma"]
    n_ap = cfg["n_ap"]
    ap_off = cfg["ap_off"]
    S_ap = cfg["S_ap"]
    calls = cfg["calls"]
    S_tot = cfg["S_tot"]
    S_ap_tot = sum(S_ap)
    ap_base = [S_dma, S_dma + S_ap[0], S_dma + S_ap[0] + S_ap[1]]

    MAXSUB_G = max(
        sum(nsub_dma[t] for t in range(g * GT, min((g + 1) * GT, NT)))
        for g in range(NGRP))

    TRD = F32R if TR_F32R else F32

    with tile.TileContext(nc) as tc, ExitStack() as ctx:
        # ---------------- persistent pool ----------------
        pp = ctx.enter_context(tc.tile_pool(name="pers", bufs=1))
        xx = pp.tile([128, VP], BF16, tag="xx")
        oh_ap = pp.tile([128, max(1, S_ap_tot), 128], FP8, tag="ohap")
        srcw_t = pp.tile([128, max(1, S_dma) * 8], I16, tag="srcw")
        apw0_t = pp.tile([128, max(1, S_ap[0]) * 8], I16, tag="apw0")
        apw1_t = pp.tile([128, max(1, S_ap[1]) * 8], I16, tag="apw1")
        apw2_t = pp.tile([128, max(1, S_ap[2]) * 8], I16, tag="apw2")
        apw_t = [apw0_t, apw1_t, apw2_t]
        dstloc_t = pp.tile([128, S_tot, 1], BF16, tag="dstloc")
        iota_t = pp.tile([128, 1, 128], BF16, tag="iota")
        ident_t = pp.tile([128, 128], F32, tag="ident")
        w0_t = pp.tile([128, 7 * 128], BF16, tag="w0")
        w1_t = pp.tile([128, 7 * 128], BF16, tag="w1")
        g0m_t = pp.tile([128, 2 * 128], BF16, tag="g0m")
        g0v_t = pp.tile([3, 256], F32, tag="g0v")
        offw_t = pp.tile([128, 3], BF16, tag="offw")
        ones_t = pp.tile([1, 512], F32, tag="ones")
        erow_t = pp.tile([1, 256], F32, tag="erow")
        encc_t = pp.tile([128, 2], F32, tag="encc")

        nc.sync.dma_start(srcw_t[:], ap["srcw"][:])
        for w in range(NW):
            nc.sync.dma_start(apw_t[w][:], ap[f"apw{w}"][:])
        nc.sync.dma_start(
            dstloc_t[:], ap["dstloc"].rearrange("p (s o) -> p s o", o=1))
        nc.sync.dma_start(iota_t[:].rearrange("p o d -> p (o d)"), ap["iota"][:])
        nc.sync.dma_start(ident_t[:], ap["ident"][:])
        nc.sync.dma_start(w0_t[:], ap["gw0"][:])
        nc.sync.dma_start(w1_t[:], ap["gw1"][:])
        nc.sync.dma_start(g0m_t[:, 0:128], ap["g0w0m"][:])
        nc.sync.dma_start(g0m_t[:, 128:256], ap["g0w1m"][:])
        nc.sync.dma_start(g0v_t[:, 0:128], ap["g0w0v"][:])
        nc.sync.dma_start(g0v_t[:, 128:256], ap["g0w1v"][:])
        nc.sync.dma_start(offw_t[:], ap["offw"][:])
        nc.vector.memset(ones_t[:], 1.0)
        nc.sync.dma_start(encc_t[:], ap["encc"][:])

        # ap one-hots, built once (fp8, resident)
        if S_ap_tot:
            nc.vector.tensor_tensor(
                out=oh_ap[:, :S_ap_tot, :],
                in0=dstloc_t[:, S_dma:S_tot, :]
                .to_broadcast([128, S_ap_tot, 128]),
                in1=iota_t[:].to_broadcast([128, S_ap_tot, 128]),
                op=mybir.AluOpType.is_equal)

        with ExitStack() as sctx:
            # ---------------- phase 1: sampling ----------------
            sp = sctx.enter_context(tc.tile_pool(name="samp", bufs=1))
            spf = sctx.enter_context(tc.tile_pool(name="sampf", bufs=3))
            spw = sctx.enter_context(tc.tile_pool(name="sampw", bufs=2))
            spp = sctx.enter_context(
                tc.tile_pool(name="sampps", bufs=2, space="PSUM"))
            spp2 = sctx.enter_context(
                tc.tile_pool(name="sampps2", bufs=2, space="PSUM"))

            g0e_t = sp.tile([128, 4 * 128], F32, tag="g0e")
            nc.sync.dma_start(
                g0e_t[:, 0:256].rearrange("p (c h) -> p c h", h=128),
                ap["g0w0e"].rearrange("(c p) h -> p c h", p=128))
            nc.sync.dma_start(
                g0e_t[:, 256:512].rearrange("p (c h) -> p c h", h=128),
                ap["g0w1e"].rearrange("(c p) h -> p c h", p=128))
            for k in range(2):
                pe = spp2.tile([1, 128], F32, tag="pe")
                for cchunk in range(2):
                    nc.tensor.matmul(
                        out=pe[:],
                        lhsT=encc_t[:, cchunk:cchunk + 1],
                        rhs=g0e_t[:, k * 256 + cchunk * 128:
                                  k * 256 + cchunk * 128 + 128],
                        start=(cchunk == 0), stop=(cchunk == 1))
                nc.scalar.activation(erow_t[:, k * 128:(k + 1) * 128], pe[:],
                                     AF.Copy)

            g_sb = sp.tile([128, NGT_ * 128], BF16, tag="gsb")
            for mi, (C, Wm) in enumerate(MAPS):
                HW = Wm * Wm
                ncc = C // 128
                bw_t = spf.tile([128, 16 * 128], BF16, tag="bw")
                nc.sync.dma_start(
                    bw_t[:, :ncc * 128].rearrange("p (c h) -> p c h", h=128),
                    ap["bw"].rearrange("(c p) h -> p c h", p=128)
                    [:, CH_OFF[mi] // 128:CH_OFF[mi] // 128 + ncc, :])
                fm_t = sp.tile([128, 2 * 3136], BF16, tag="fm")
                nc.sync.dma_start(
                    fm_t[:, :ncc * HW].rearrange("p (c hw) -> p c hw", c=ncc),
                    ap[f"f{mi+1}"].rearrange("(c p) hw -> p c hw", p=128))
                for t in range(ntile_map[mi]):
                    p0 = t * 128
                    pcnt = min(128, HW - p0)
                    pg = spp2.tile([128, 128], F32, tag="pg")
                    for cc in range(ncc):
                        nc.tensor.matmul(
                            out=pg[:pcnt, :],
                            lhsT=fm_t[:, cc * HW + p0:cc * HW + p0 + pcnt],
                            rhs=bw_t[:, cc * 128:cc * 128 + 128],
                            start=(cc == 0), stop=(cc == ncc - 1))
                    gt = g_off[mi] + t
                    nc.scalar.activation(
                        g_sb[:pcnt, gt * 128:gt * 128 + 128], pg[:pcnt, :],
                        AF.Copy)

            npc = sum(np_list)
            for c in range(NVCH):
                ps = spp.tile([128, 512], F32, tag="ps")
                pairs_c = []
                for mi in range(4):
                    for t in sched[mi][c]:
                        pairs_c.append((mi, t))
                half = (npc + 1) // 2
                wts = []
                for hb in range(2):
                    k0, k1 = hb * half, min((hb + 1) * half, npc)
                    wt = spw.tile([128, half, 512], BF16, tag="wsc")
                    nc.sync.dma_start(
                        wt[:, :k1 - k0, :],
                        ap["wsc"].rearrange("(k p) h -> p k h", p=128)
                        [:, c * npc + k0:c * npc + k1, :])
                    wts.append(wt)
                for k, (mi, t) in enumerate(pairs_c):
                    HW = MAPS[mi][1] ** 2
                    pcnt = min(128, HW - t * 128)
                    gt = g_off[mi] + t
                    nc.tensor.matmul(
                        out=ps[:],
                        lhsT=g_sb[:pcnt, gt * 128:gt * 128 + 128],
                        rhs=wts[k // half][:pcnt, k % half, :],
                        start=(k == 0), stop=(k == len(pairs_c) - 1))
                nc.scalar.activation(xx[:, c * 512:(c + 1) * 512], ps[:],
                                     AF.Relu)

        # ---------------- phase 2: graph conv layers ----------------
        lpool = ctx.enter_context(tc.tile_pool(name="h1c", bufs=1))
        h1c = lpool.tile([128, VP], F32, tag="h1c")
        lp = ctx.enter_context(tc.tile_pool(name="msg", bufs=2))
        lph = ctx.enter_context(tc.tile_pool(name="hst", bufs=2))
        lpv = ctx.enter_context(tc.tile_pool(name="vv", bufs=1))
        apb = ctx.enter_context(tc.tile_pool(name="apbuf", bufs=2))
        trp = ctx.enter_context(tc.tile_pool(name="trmsg", bufs=3))
        ohd = ctx.enter_context(tc.tile_pool(name="ohdma", bufs=2))
        pst = ctx.enter_context(tc.tile_pool(name="pstr", bufs=2, space="PSUM"))
        psh = ctx.enter_context(tc.tile_pool(name="psh", bufs=2, space="PSUM"))
        psx = ctx.enter_context(tc.tile_pool(name="psx", bufs=2, space="PSUM"))

        def emit_h1_rows(l, c0, nt4, h1_writes):
            """h1 rows for layer l, tiles [c0, c0+nt4) -> h1d2[l % 2]."""
            h1d = h1d2[l % 2]
            ph = psh.tile([128, 512], F32, tag="ph")
            if l == 0:
                vv = lpv.tile([3, 8 * 128], F32, tag="vt")
                nc.sync.dma_start(
                    vv[:, :nt4 * 128],
                    ap["vertsT"][:, c0 * 128:(c0 + nt4) * 128])
            for ti in range(nt4):
                t = c0 + ti
                sl = slice(ti * 128, (ti + 1) * 128)
                if l == 0:
                    nc.tensor.matmul(
                        out=ph[:, sl], lhsT=xx[:, t * 128:(t + 1) * 128],
                        rhs=g0m_t[:, 128:256], start=True, stop=False)
                    nc.tensor.matmul(
                        out=ph[:, sl], lhsT=vv[:, ti * 128:(ti + 1) * 128],
                        rhs=g0v_t[:, 128:256], start=False, stop=False)
                    nc.tensor.matmul(
                        out=ph[:, sl], lhsT=ones_t[:, 0:128],
                        rhs=erow_t[:, 128:256], start=False, stop=True)
                else:
                    nc.tensor.matmul(
                        out=ph[:, sl], lhsT=xx[:, t * 128:(t + 1) * 128],
                        rhs=w1_t[:, (l - 1) * 128:l * 128],
                        start=True, stop=True)
            hst = lph.tile([128, 512], BF16, tag="hst")
            nc.scalar.activation(hst[:, :nt4 * 128], ph[:, :nt4 * 128],
                                 AF.Copy)
            h1_writes.append(nc.sync.dma_start(
                h1d.rearrange("(n p) c -> p n c", p=128)[:, c0:c0 + nt4, :],
                hst[:, :nt4 * 128].rearrange("p (n c) -> p n c", c=128)))

        def emit_h1_cols(l, c):
            """h1 column chunk c (512 cols) for layer l -> h1c (f32)."""
            c0 = c * 512
            cw = 512
            ph = psh.tile([128, 512], F32, tag="ph")
            if l == 0:
                vv = lpv.tile([3, 8 * 128], F32, tag="vt")
                nc.sync.dma_start(vv[:, :cw], ap["vertsT"][:, c0:c0 + cw])
                nc.tensor.matmul(
                    out=ph[:], lhsT=g0m_t[:, 128:256],
                    rhs=xx[:, c0:c0 + cw], start=True, stop=False)
                nc.tensor.matmul(
                    out=ph[:], lhsT=g0v_t[:, 128:256],
                    rhs=vv[:, :cw], start=False, stop=False)
                nc.tensor.matmul(
                    out=ph[:], lhsT=erow_t[:, 128:256],
                    rhs=ones_t[:, :cw], start=False, stop=True)
            else:
                nc.tensor.matmul(
                    out=ph[:], lhsT=w1_t[:, (l - 1) * 128:l * 128],
                    rhs=xx[:, c0:c0 + cw], start=True, stop=True)
            nc.scalar.activation(h1c[:, c0:c0 + cw], ph[:], AF.Copy)

        CPW = WSZ // 512  # h1c chunks per window (7)

        def _layer(l, h1_writes, last_layer):
            """Scatter groups for layer l; h1 for layer l+1 is emitted inside
            (pipelined). Returns layer l+1's h1_writes list."""
            h1d = h1d2[l % 2]
            h1_writes_next = []
            trmsg_tiles = [[None] * len(calls[w]) for w in range(NW)]
            copy_flip = [0]

            def emit_call(w, k):
                s0, s1, _t0, _t1, _gs, _ge = calls[w][k]
                ns = s1 - s0
                buf = apb.tile([128, CALL_SUBS * 128], F32, tag="apbuf")
                nc.gpsimd.ap_gather(
                    out_ap=buf[:, :ns * 128],
                    in_ap=h1c[:, w * WSZ:(w + 1) * WSZ],
                    idxs_ap=apw_t[w][:, s0 * 8:s1 * 8],
                    channels=128, num_elems=WSZ, d=1, num_idxs=ns * 128)
                tr = trp.tile([128, CALL_SUBS, 128], BF16, tag="trmsg")
                trmsg_tiles[w][k] = (tr, s0)
                for j4 in range(0, ns, 4):
                    jn = min(4, ns - j4)
                    pt = pst.tile([128, 512], F32, tag="pt")
                    for j in range(jn):
                        src_sl = buf[:, (j4 + j) * 128:(j4 + j + 1) * 128]
                        id_sl = ident_t[:]
                        out_sl = pt[:, j * 128:(j + 1) * 128]
                        if TR_F32R:
                            src_sl = src_sl.bitcast(F32R)
                            id_sl = id_sl.bitcast(F32R)
                            out_sl = out_sl.bitcast(F32R)
                        nc.tensor.transpose(out_sl, src_sl, id_sl)
                    dst_sl = tr[:, j4:j4 + jn, :].rearrange("p s o -> p (s o)")
                    if copy_flip[0] % 2 == 0:
                        nc.vector.tensor_copy(dst_sl, pt[:, :jn * 128])
                    else:
                        nc.scalar.activation(dst_sl, pt[:, :jn * 128], AF.Copy)
                    copy_flip[0] += 1

            next_call = [0] * NW
            pending_cols = [] if last_layer else list(range(NVCH))

            for g in range(NGRP):
                t_lo = g * GT
                t_hi = min((g + 1) * GT, NT)
                for w in range(NW):
                    while (next_call[w] < len(calls[w])
                           and calls[w][next_call[w]][4] <= g):
                        emit_call(w, next_call[w])
                        next_call[w] += 1

                W_ = (t_hi - t_lo) * 128
                px = psx.tile([128, GT * 128], F32, tag="px")

                # DMA-half messages + one-hots for this group
                s0, s1 = sub_off[t_lo], sub_off[t_hi]
                ng = s1 - s0
                if ng > 0:
                    msg = lp.tile([128, MAXSUB_G, 128], BF16, tag="msg")
                    gi = nc.gpsimd.dma_gather(
                        out_ap=msg[:, :ng, :],
                        in_ap=h1d[:],
                        idxs_ap=srcw_t[:, s0 * 8:s1 * 8],
                        num_idxs=ng * 128,
                        num_idxs_reg=ng * 128,
                        elem_size=HID,
                        single_packet=False,
                    )
                    for wi in h1_writes:
                        tile.add_dep_helper(gi.ins, wi.ins,
                                            reason="h1 RAW: gather after write")
                    ohg = ohd.tile([128, MAXSUB_G, 128], FP8, tag="ohg")
                    nc.vector.tensor_tensor(
                        out=ohg[:, :ng, :],
                        in0=dstloc_t[:, s0:s1, :].to_broadcast([128, ng, 128]),
                        in1=iota_t[:].to_broadcast([128, ng, 128]),
                        op=mybir.AluOpType.is_equal)

                # collect accumulating matmuls; psum-bank-sized segments get
                # their own start/stop
                mms = []  # entries: (seg_id, kwargs)
                if l == 0:
                    vv2 = lpv.tile([3, GT * 128], F32, tag="vt2")
                    nc.sync.dma_start(
                        vv2[:, :W_], ap["vertsT"][:, t_lo * 128:t_hi * 128])
                    for seg in range(0, W_, 512):
                        sw = min(512, W_ - seg)
                        c0 = t_lo * 128 + seg
                        mms.append((seg // 512,
                                    dict(out=px[:, seg:seg + sw],
                                         lhsT=g0m_t[:, 0:128],
                                         rhs=xx[:, c0:c0 + sw])))
                        mms.append((seg // 512,
                                    dict(out=px[:, seg:seg + sw],
                                         lhsT=g0v_t[:, 0:128],
                                         rhs=vv2[:, seg:seg + sw])))
                        mms.append((seg // 512,
                                    dict(out=px[:, seg:seg + sw],
                                         lhsT=erow_t[:, 0:128],
                                         rhs=ones_t[:, :sw])))
                else:
                    for seg in range(0, W_, 512):
                        sw = min(512, W_ - seg)
                        c0 = t_lo * 128 + seg
                        mms.append((seg // 512,
                                    dict(out=px[:, seg:seg + sw],
                                         lhsT=w0_t[:, (l - 1) * 128:l * 128],
                                         rhs=xx[:, c0:c0 + sw])))
                for ti in range(t_hi - t_lo):
                    t = t_lo + ti
                    osl = slice(ti * 128, (ti + 1) * 128)
                    for j in range(sub_off[t] - s0, sub_off[t + 1] - s0):
                        mms.append((ti * 128 // 512,
                                    dict(out=px[:, osl], lhsT=msg[:, j, :],
                                         rhs=ohg[:, j, :])))
                    for w in range(NW):
                        na = n_ap[t][w]
                        if na == 0:
                            continue
                        kk = next(
                            i for i, c in enumerate(calls[w])
                            if c[2] <= t < c[3])
                        tr, trs0 = trmsg_tiles[w][kk]
                        for j in range(na):
                            s_loc = ap_off[t][w] - trs0 + j
                            s_ap = ap_base[w] - S_dma + ap_off[t][w] + j
                            mms.append((ti * 128 // 512,
                                        dict(out=px[:, osl],
                                             lhsT=tr[:, s_loc, :],
                                             rhs=oh_ap[:, s_ap, :])))
                first_of = {}
                last_of = {}
                for i, (sg, _kw) in enumerate(mms):
                    first_of.setdefault(sg, i)
                    last_of[sg] = i
                for i, (sg, kw) in enumerate(mms):
                    nc.tensor.matmul(start=(first_of[sg] == i),
                                     stop=(last_of[sg] == i),
                                     skip_group_check=True, **kw)
                nc.scalar.activation(xx[:, t_lo * 128:t_hi * 128], px[:, :W_],
                                     AF.Relu)

                # ---- pipelined layer-(l+1) h1 production ----
                if not last_layer:
                    for c0 in range(t_lo, t_hi, 4):
                        emit_h1_rows(l + 1, c0, min(4, t_hi - c0),
                                     h1_writes_next)
                    still = []
                    for c in pending_cols:
                        w = c // CPW
                        src_g = (c * 4 + 3) // GT
                        if next_call[w] == len(calls[w]) and src_g <= g:
                            emit_h1_cols(l + 1, c)
                        else:
                            still.append(c)
                    pending_cols = still
            assert not pending_cols
            return h1_writes_next

        for _rep in range(repeat):
            h1w = []
            for c0 in range(0, NT, 4):
                emit_h1_rows(0, c0, min(4, NT - c0), h1w)
            for c in range(NVCH):
                emit_h1_cols(0, c)
            for l in range(nlayers):
                h1w = _layer(l, h1w, l == nlayers - 1)

        # ---------------- output ----------------
        OB = 12
        for t0 in range(0, NT, OB):
            tb = min(OB, NT - t0)
            ost = lph.tile([128, OB * 3], F32, tag="ost")
            for ti in range(tb):
                t = t0 + ti
                po = psh.tile([128, 512], F32, tag="ph")
                nc.tensor.matmul(out=po[:, :3],
                                 lhsT=xx[:, t * 128:(t + 1) * 128],
                                 rhs=offw_t[:], start=True, stop=True)
                nc.scalar.activation(ost[:, ti * 3:(ti + 1) * 3], po[:, :3],
                                     AF.Copy)
            nc.sync.dma_start(
                out.rearrange("(n p) c -> p n c", p=128)[:, t0:t0 + tb, :],
                ost[:, :tb * 3].rearrange("p (n c) -> p n c", c=3))

    nc.compile()
    return nc


_CACHE = {}


def kernel(**inputs) -> np.ndarray:
    cfg, per_core, post = _prep(inputs)
    key = (cfg["npair"], cfg["S_tot"], cfg["S_dma"],
           tuple(cfg["np_list"]), str(cfg["calls"]))
    if key not in _CACHE:
        _CACHE[key] = _build(cfg, per_core[0])
    nc = _CACHE[key]
    res = run_bass_kernel_spmd(nc, per_core, list(range(B)))
    outs = np.empty((B, V, 3), np.float32)
    for m in range(B):
        rows = res.results[m]["out"]
        outs[m] = rows[post["slot_of"][m][np.arange(V)]]
    return outs.reshape(B * V, 3)


if __name__ == "__main__":
    pass


# revision 18
# speedup vs baseline: 1.0184x; 1.0184x over previous
"""Trainium2 Bass kernel for DeformationNetworkGraphConvolutionalFullRes.

Full (unsharded) inputs in, full output out. Data-parallel over the 4 meshes:
core m processes mesh m. Inside each core:

  - vert_align sampling as (S @ F) @ W == S @ (F @ W): per feature map,
    F[C,HW] @ Wslice[C,128] -> G[HW,128] (bf16), then the sparse bilinear
    operator S applied as dense [128px, 512vert] bf16 blocks on the
    TensorEngine, accumulating in PSUM. Vertices pre-sorted by image cell.
  - Each GraphConv layer routes its 61440 directed-edge messages through TWO
    independent engines in parallel:
      * DMA half: h1 rows written to HBM, messages pulled with dma_gather
        in dst-sorted order (DMA engines).
      * ap half: h1 kept as f32 columns in SBUF (three [128, 3584] windows);
        gpsimd.ap_gather selects message columns (Pool engine), PE
        transposes them to row form.
    Both halves are scatter-added per dst tile with one-hot matmuls
    (ap one-hots built once in fp8 and kept SBUF-resident; DMA-half
    one-hots rebuilt per group on DVE), accumulating in PSUM on top of
    h0 = W0^T x (+ rank-1 image-encoding term at layer 0); ReLU writes
    the bf16 column-form activations in place.
"""

import ml_dtypes
import numpy as np
from contextlib import ExitStack

import concourse.bass as bass
import concourse.tile as tile
from concourse import bacc, mybir
from concourse.bass_utils import run_bass_kernel_spmd

# ---------------- problem constants (hardcoded per spec) ----------------
B = 4
V = 10242
E_PER = 30720
HID = 128
MAPS = [(256, 56), (512, 28), (1024, 14), (2048, 7)]  # (C, H==W)
CH_OFF = [0, 256, 768, 1792, 3840]

VP = 10752            # padded vertex count: 84 tiles of 128
NT = VP // 128        # 84 vertex tiles
NVCH = VP // 512      # 21 vertex chunks (sampling)
NW = 3                # ap-gather source windows
WSZ = VP // NW        # 3584 columns per window
VW = V // NW          # 3414 real verts per window
GT = 8                # dst tiles per scatter group
NGRP = (NT + GT - 1) // GT  # 11 groups (last has 4 tiles)
CALL_SUBS = 32        # max ap-gather call size (subchunks of 128)
AP_FRAC = 0.50        # target fraction of edges through the ap path
TR_F32R = False       # f32r rejected by BIR verifier (needs rounded input)

F32 = mybir.dt.float32
F32R = mybir.dt.float32r
BF16 = mybir.dt.bfloat16
FP8 = mybir.dt.float8e4
I16 = mybir.dt.int16
AF = mybir.ActivationFunctionType


def _corners(grid, W):
    """grid [V,2] in [-1,1] -> list of (pix_idx int64, weight f32) per corner."""
    x = (grid[:, 0] + 1.0) * 0.5 * (W - 1)
    y = (grid[:, 1] + 1.0) * 0.5 * (W - 1)
    x0f, y0f = np.floor(x), np.floor(y)
    wx1, wy1 = (x - x0f).astype(np.float32), (y - y0f).astype(np.float32)
    wx0, wy0 = 1.0 - wx1, 1.0 - wy1
    x0 = np.clip(x0f, 0, W - 1).astype(np.int64)
    x1 = np.clip(x0f + 1, 0, W - 1).astype(np.int64)
    y0 = np.clip(y0f, 0, W - 1).astype(np.int64)
    y1 = np.clip(y0f + 1, 0, W - 1).astype(np.int64)
    return [
        (y0 * W + x0, wy0 * wx0),
        (y0 * W + x1, wy0 * wx1),
        (y1 * W + x0, wy1 * wx0),
        (y1 * W + x1, wy1 * wx1),
    ]


def _wrap16(idx):
    """int array [n] (n % 16 == 0) -> [128, n/16] wrapped+replicated for the
    8 Q7 cores (idx i at (i%16, i//16))."""
    return np.tile(idx.reshape(-1, 16).T, (8, 1)).astype(np.int16)


def _prep(inputs):
    """Host-side restructuring. Returns (cfg, per_core_aux_list, post)."""
    feats = [inputs["feat1"], inputs["feat2"], inputs["feat3"], inputs["feat4"]]
    av = np.asarray(inputs["aligned_verts"], np.float32)
    verts = np.asarray(inputs["verts_packed"], np.float32)
    enc = np.asarray(inputs["image_enc"], np.float32)
    edges = np.asarray(inputs["edges"], np.int64)

    for bn in ["bottleneck_b", "g0_b0", "g0_b1", "off_b"]:
        assert not np.any(np.asarray(inputs[bn])), f"{bn} nonzero: unsupported"
    assert not np.any(np.asarray(inputs["gb0"])) and not np.any(
        np.asarray(inputs["gb1"])
    ), "gb nonzero: unsupported"

    # per-mesh vertex sort (by finest-map cell); cell-sorted order split into
    # NW windows of VW real verts; window w occupies slots [w*WSZ, w*WSZ+VW),
    # the rest of each window is zero padding.
    sigmas, slot_of = [], []
    corners_all = []
    for m in range(B):
        grid = av[m, :, :2]
        cs = _corners(grid, MAPS[0][1])
        key = cs[0][0]
        sigma = np.argsort(key, kind="stable")
        slot = np.full(V, -1, np.int64)
        for w in range(NW):
            slot[sigma[w * VW:(w + 1) * VW]] = w * WSZ + np.arange(VW)
        sigmas.append(sigma)
        slot_of.append(slot)
        corners_all.append([_corners(grid, Wm) for (_, Wm) in MAPS])

    vert_at = []
    for m in range(B):
        va = np.full(VP, -1, np.int64)
        va[slot_of[m][np.arange(V)]] = np.arange(V)
        vert_at.append(va)

    # sampling schedule
    ntile_map = [(Wm * Wm + 127) // 128 for (_, Wm) in MAPS]
    g_off = np.cumsum([0] + ntile_map)
    sched = []
    for mi in range(4):
        per_c = []
        for c in range(NVCH):
            lo, hi = c * 512, (c + 1) * 512
            tiles = set()
            for m in range(B):
                vs = vert_at[m][lo:hi]
                vs = vs[vs >= 0]
                if len(vs):
                    for (pix, _w) in corners_all[m][mi]:
                        tiles.update(np.unique(pix[vs] // 128).tolist())
            per_c.append(sorted(tiles) if tiles else [0])
        np_m = max(len(t) for t in per_c)
        per_c = [t + [t[0]] * (np_m - len(t)) for t in per_c]
        sched.append(per_c)
    np_list = [len(sched[mi][0]) for mi in range(4)]
    npair = sum(np_list) * NVCH

    # graph structure ------------------------------------------------------
    # directed edges in slot space, sorted by (dst tile, src window)
    per_mesh_edges = []
    cnt_tw = np.zeros((B, NT, NW), np.int64)
    for m in range(B):
        e = edges[m * E_PER:(m + 1) * E_PER] - m * V
        a = slot_of[m][e[:, 0]]
        b_ = slot_of[m][e[:, 1]]
        dst = np.concatenate([a, b_])
        src = np.concatenate([b_, a])
        win = src // WSZ
        order = np.lexsort((src, win, dst // 128))
        dst, src, win = dst[order], src[order], win[order]
        per_mesh_edges.append((dst, src, win))
        tl = dst // 128
        for t in range(NT):
            sel = tl == t
            for w in range(NW):
                cnt_tw[m, t, w] = np.sum(sel & (win == w))

    # ap routing: n_ap[t][w] full subchunks through the ap path (shared)
    min_cnt = cnt_tw.min(axis=0)  # [NT, NW]
    n_ap = np.minimum(min_cnt // 128, 2).astype(np.int64)
    target_slots = int(AP_FRAC * 2 * E_PER)
    cur = int(n_ap.sum() * 128)
    order2 = np.argsort(min_cnt.reshape(-1) % 128)
    for idx in order2:
        if cur <= target_slots:
            break
        t, w = divmod(int(idx), NW)
        if n_ap[t, w] > 0:
            n_ap[t, w] -= 1
            cur -= 128
    # DMA-half subchunk counts (shared)
    rem = cnt_tw - n_ap[None] * 128
    rem_t = rem.sum(axis=2)
    nsub_dma = np.maximum(0, -(-rem_t.max(axis=0) // 128))
    sub_off = np.concatenate([[0], np.cumsum(nsub_dma)]).astype(int)
    S_dma = int(sub_off[-1])

    # ap stream layout per window: subchunk ranges per tile
    ap_off = np.zeros((NT + 1, NW), np.int64)
    for w in range(NW):
        ap_off[1:, w] = np.cumsum(n_ap[:, w])
    S_ap = [int(ap_off[NT, w]) for w in range(NW)]
    S_ap_tot = sum(S_ap)

    # ap gather call partition per stream: whole-GROUP tile ranges (so that
    # successive trmsg-buffer tenants have strictly disjoint group ranges),
    # each call <= CALL_SUBS subchunks.
    calls = []  # per w: list of (sub0, sub1, t0, t1, g_start, g_end)
    for w in range(NW):
        cl = []
        g0 = 0
        while g0 < NGRP:
            g1 = g0
            while (g1 < NGRP
                   and ap_off[min((g1 + 1) * GT, NT), w]
                   - ap_off[g0 * GT, w] <= CALL_SUBS):
                g1 += 1
            assert g1 > g0, f"group {g0} stream {w} exceeds CALL_SUBS"
            t0, t1 = g0 * GT, min(g1 * GT, NT)
            s0, s1 = int(ap_off[t0, w]), int(ap_off[t1, w])
            if s1 > s0:
                cl.append((s0, s1, t0, t1, g0, g1 - 1))
            g0 = g1
        calls.append(cl)

    # trmsg rotation safety (shared pool of 3 bufs, merged emission order):
    # emission at group g_start must come after all consumers of the buffer's
    # previous tenant (g_end of the call 3 positions earlier in merged order).
    merged = []
    for w in range(NW):
        for k, c in enumerate(calls[w]):
            merged.append((c[4], c[5], w, k))
    merged.sort()
    for i in range(3, len(merged)):
        assert merged[i][0] > merged[i - 3][1], (
            f"trmsg rotation hazard: {merged[i]} vs {merged[i-3]}")

    S_tot = S_dma + S_ap_tot

    MAXCALL = max(max((c[1] - c[0]) for c in cl) for cl in calls if cl)
    cfg = {"MAXCALL": MAXCALL,
           "sched": sched, "np_list": np_list, "g_off": g_off.tolist(),
           "ntile_map": ntile_map, "nsub_dma": nsub_dma.tolist(),
           "sub_off": sub_off.tolist(), "S_dma": S_dma,
           "n_ap": n_ap.tolist(), "ap_off": ap_off.tolist(),
           "S_ap": S_ap, "calls": calls, "S_tot": S_tot, "npair": npair}

    # ---------------- per-core tables ----------------
    per_core = []
    for m in range(B):
        dst, src, win = per_mesh_edges[m]
        ap_idx = [np.zeros(max(16, S_ap[w] * 128), np.int64) for w in range(NW)]
        ap_dl = [np.zeros(S_ap[w] * 128, np.int64) for w in range(NW)]
        src_slots = np.zeros(max(16, S_dma * 128), np.int64)
        dl_slots = np.full(S_dma * 128, -1, np.int64)
        pos = 0
        for t in range(NT):
            dma_d, dma_s = [], []
            for w in range(NW):
                c = int(cnt_tw[m, t, w])
                d_, s_ = dst[pos:pos + c], src[pos:pos + c]
                na = int(n_ap[t, w]) * 128
                o = int(ap_off[t, w]) * 128
                ap_idx[w][o:o + na] = s_[:na] - w * WSZ
                ap_dl[w][o:o + na] = d_[:na] - t * 128
                dma_d.append(d_[na:])
                dma_s.append(s_[na:])
                pos += c
            d_ = np.concatenate(dma_d)
            s_ = np.concatenate(dma_s)
            so = sub_off[t] * 128
            src_slots[so:so + len(s_)] = s_
            dl_slots[so:so + len(d_)] = d_ - t * 128
        assert pos == 2 * E_PER

        dl_all = np.concatenate([dl_slots] + ap_dl)
        dl_tab = dl_all.reshape(S_tot, 128).T.copy().astype(ml_dtypes.bfloat16)

        srcw = _wrap16(src_slots)
        apw = [_wrap16(ap_idx[w]) for w in range(NW)]

        # sampling blocks ---------------------------------------------------
        npc = sum(np_list)
        wsc = np.zeros((npair, 128, 512), np.float32)
        pi = 0
        for c in range(NVCH):
            lo = c * 512
            vs_all = vert_at[m][lo:lo + 512]
            jj = np.nonzero(vs_all >= 0)[0]
            for mi in range(4):
                seen = set()
                for t in sched[mi][c]:
                    blk = wsc[pi]
                    if t not in seen and len(jj):
                        seen.add(t)
                        for (pix, w_) in corners_all[m][mi]:
                            px = pix[vs_all[jj]]
                            sel = (px >= t * 128) & (px < (t + 1) * 128)
                            jj2 = jj[sel]
                            np.add.at(blk, (pix[vs_all[jj2]] - t * 128, jj2),
                                      w_[vs_all[jj2]])
                    pi += 1
        assert pi == npair

        vt = np.zeros((3, VP), np.float32)
        vslots = slot_of[m][np.arange(V)]
        vt[:, vslots] = verts[m * V:(m + 1) * V].T

        bf = ml_dtypes.bfloat16
        aux = {
            "f1": feats[0][m].reshape(256, -1).astype(bf),
            "f2": feats[1][m].reshape(512, -1).astype(bf),
            "f3": feats[2][m].reshape(1024, -1).astype(bf),
            "f4": feats[3][m].reshape(2048, -1).astype(bf),
            "bw": np.asarray(inputs["bottleneck_w"], np.float32).astype(bf),
            "wsc": wsc.reshape(npair * 128, 512).astype(bf),
            "srcw": np.ascontiguousarray(srcw),
            "apw0": np.ascontiguousarray(apw[0]),
            "apw1": np.ascontiguousarray(apw[1]),
            "apw2": np.ascontiguousarray(apw[2]),
            "dstloc": np.ascontiguousarray(dl_tab),
            "iota": np.tile(np.arange(128, dtype=bf), (128, 1)),
            "ident": np.eye(128, dtype=np.float32),
            "vertsT": vt,
            "encc": enc[m].reshape(2, 128).T.copy(),
            "g0w0m": np.asarray(inputs["g0_w0"][:128], np.float32).astype(bf),
            "g0w1m": np.asarray(inputs["g0_w1"][:128], np.float32).astype(bf),
            "g0w0v": np.asarray(inputs["g0_w0"][128:131], np.float32),
            "g0w1v": np.asarray(inputs["g0_w1"][128:131], np.float32),
            "g0w0e": np.ascontiguousarray(
                np.asarray(inputs["g0_w0"][131:387], np.float32)),
            "g0w1e": np.ascontiguousarray(
                np.asarray(inputs["g0_w1"][131:387], np.float32)),
            "gw0": np.ascontiguousarray(
                np.asarray(inputs["gw0"], np.float32).transpose(1, 0, 2)
                .reshape(128, 7 * 128)).astype(bf),
            "gw1": np.ascontiguousarray(
                np.asarray(inputs["gw1"], np.float32).transpose(1, 0, 2)
                .reshape(128, 7 * 128)).astype(bf),
            "offw": np.asarray(inputs["off_w"], np.float32).astype(bf),
        }
        per_core.append(aux)

    post = {"slot_of": slot_of}
    return cfg, per_core, post


def _build(cfg, shapes, nlayers=8, repeat=1):
    """Build the SPMD Bass program (same instruction stream for all cores)."""
    nc = bacc.Bacc("TRN2", target_bir_lowering=False, debug=False, num_devices=B)
    ap = {}
    for name, arr in shapes.items():
        ap[name] = nc.dram_tensor(
            name, list(arr.shape), mybir.dt.from_np(arr.dtype),
            kind="ExternalInput").ap()
    out = nc.dram_tensor("out", [VP, 3], F32, kind="ExternalOutput").ap()
    h1d2 = [nc.dram_tensor("h1da", [VP, HID], BF16).ap(),
            nc.dram_tensor("h1db", [VP, HID], BF16).ap()]

    sched = cfg["sched"]
    np_list = cfg["np_list"]
    g_off = cfg["g_off"]
    ntile_map = cfg["ntile_map"]
    NGT_ = g_off[4]
    nsub_dma = cfg["nsub_dma"]
    sub_off = cfg["sub_off"]
    S_dma = cfg["S_dma"]
    n_ap = cfg["n_ap"]
    ap_off = cfg["ap_off"]
    S_ap = cfg["S_ap"]
    calls = cfg["calls"]
    S_tot = cfg["S_tot"]
    S_ap_tot = sum(S_ap)
    MAXCALL = cfg["MAXCALL"]
    ap_base = [S_dma, S_dma + S_ap[0], S_dma + S_ap[0] + S_ap[1]]

    MAXSUB_G = max(
        sum(nsub_dma[t] for t in range(g * GT, min((g + 1) * GT, NT)))
        for g in range(NGRP))

    TRD = F32R if TR_F32R else F32

    with tile.TileContext(nc) as tc, ExitStack() as ctx:
        # ---------------- persistent pool ----------------
        pp = ctx.enter_context(tc.tile_pool(name="pers", bufs=1))
        xx = pp.tile([128, VP], BF16, tag="xx")
        oh_ap = pp.tile([128, max(1, S_ap_tot), 128], FP8, tag="ohap")
        srcw_t = pp.tile([128, max(1, S_dma) * 8], I16, tag="srcw")
        apw0_t = pp.tile([128, max(1, S_ap[0]) * 8], I16, tag="apw0")
        apw1_t = pp.tile([128, max(1, S_ap[1]) * 8], I16, tag="apw1")
        apw2_t = pp.tile([128, max(1, S_ap[2]) * 8], I16, tag="apw2")
        apw_t = [apw0_t, apw1_t, apw2_t]
        dstloc_t = pp.tile([128, S_tot, 1], BF16, tag="dstloc")
        iota_t = pp.tile([128, 1, 128], BF16, tag="iota")
        ident_t = pp.tile([128, 128], F32, tag="ident")
        w0_t = pp.tile([128, 7 * 128], BF16, tag="w0")
        w1_t = pp.tile([128, 7 * 128], BF16, tag="w1")
        g0m_t = pp.tile([128, 2 * 128], BF16, tag="g0m")
        g0v_t = pp.tile([3, 256], F32, tag="g0v")
        offw_t = pp.tile([128, 3], BF16, tag="offw")
        ones_t = pp.tile([1, 512], BF16, tag="ones")
        erow_t = pp.tile([1, 256], BF16, tag="erow")
        encc_t = pp.tile([128, 2], F32, tag="encc")

        nc.sync.dma_start(srcw_t[:], ap["srcw"][:])
        for w in range(NW):
            nc.sync.dma_start(apw_t[w][:], ap[f"apw{w}"][:])
        nc.sync.dma_start(
            dstloc_t[:], ap["dstloc"].rearrange("p (s o) -> p s o", o=1))
        nc.sync.dma_start(iota_t[:].rearrange("p o d -> p (o d)"), ap["iota"][:])
        nc.sync.dma_start(ident_t[:], ap["ident"][:])
        nc.sync.dma_start(w0_t[:], ap["gw0"][:])
        nc.sync.dma_start(w1_t[:], ap["gw1"][:])
        nc.sync.dma_start(g0m_t[:, 0:128], ap["g0w0m"][:])
        nc.sync.dma_start(g0m_t[:, 128:256], ap["g0w1m"][:])
        nc.sync.dma_start(g0v_t[:, 0:128], ap["g0w0v"][:])
        nc.sync.dma_start(g0v_t[:, 128:256], ap["g0w1v"][:])
        nc.sync.dma_start(offw_t[:], ap["offw"][:])
        nc.vector.memset(ones_t[:], 1.0)
        nc.sync.dma_start(encc_t[:], ap["encc"][:])

        # ap one-hots, built once (fp8, resident)
        if S_ap_tot:
            nc.vector.tensor_tensor(
                out=oh_ap[:, :S_ap_tot, :],
                in0=dstloc_t[:, S_dma:S_tot, :]
                .to_broadcast([128, S_ap_tot, 128]),
                in1=iota_t[:].to_broadcast([128, S_ap_tot, 128]),
                op=mybir.AluOpType.is_equal)

        with ExitStack() as sctx:
            # ---------------- phase 1: sampling ----------------
            sp = sctx.enter_context(tc.tile_pool(name="samp", bufs=1))
            spf = sctx.enter_context(tc.tile_pool(name="sampf", bufs=3))
            spw = sctx.enter_context(tc.tile_pool(name="sampw", bufs=2))
            spp = sctx.enter_context(
                tc.tile_pool(name="sampps", bufs=2, space="PSUM"))
            spp2 = sctx.enter_context(
                tc.tile_pool(name="sampps2", bufs=2, space="PSUM"))

            g0e_t = sp.tile([128, 4 * 128], F32, tag="g0e")
            nc.sync.dma_start(
                g0e_t[:, 0:256].rearrange("p (c h) -> p c h", h=128),
                ap["g0w0e"].rearrange("(c p) h -> p c h", p=128))
            nc.sync.dma_start(
                g0e_t[:, 256:512].rearrange("p (c h) -> p c h", h=128),
                ap["g0w1e"].rearrange("(c p) h -> p c h", p=128))
            for k in range(2):
                pe = spp2.tile([1, 128], F32, tag="pe")
                for cchunk in range(2):
                    nc.tensor.matmul(
                        out=pe[:],
                        lhsT=encc_t[:, cchunk:cchunk + 1],
                        rhs=g0e_t[:, k * 256 + cchunk * 128:
                                  k * 256 + cchunk * 128 + 128],
                        start=(cchunk == 0), stop=(cchunk == 1))
                nc.scalar.activation(erow_t[:, k * 128:(k + 1) * 128], pe[:],
                                     AF.Copy)

            g_sb = sp.tile([128, NGT_ * 128], BF16, tag="gsb")
            for mi, (C, Wm) in enumerate(MAPS):
                HW = Wm * Wm
                ncc = C // 128
                bw_t = spf.tile([128, 16 * 128], BF16, tag="bw")
                nc.sync.dma_start(
                    bw_t[:, :ncc * 128].rearrange("p (c h) -> p c h", h=128),
                    ap["bw"].rearrange("(c p) h -> p c h", p=128)
                    [:, CH_OFF[mi] // 128:CH_OFF[mi] // 128 + ncc, :])
                fm_t = sp.tile([128, 2 * 3136], BF16, tag="fm")
                nc.sync.dma_start(
                    fm_t[:, :ncc * HW].rearrange("p (c hw) -> p c hw", c=ncc),
                    ap[f"f{mi+1}"].rearrange("(c p) hw -> p c hw", p=128))
                for t in range(ntile_map[mi]):
                    p0 = t * 128
                    pcnt = min(128, HW - p0)
                    pg = spp2.tile([128, 128], F32, tag="pg")
                    for cc in range(ncc):
                        nc.tensor.matmul(
                            out=pg[:pcnt, :],
                            lhsT=fm_t[:, cc * HW + p0:cc * HW + p0 + pcnt],
                            rhs=bw_t[:, cc * 128:cc * 128 + 128],
                            start=(cc == 0), stop=(cc == ncc - 1))
                    gt = g_off[mi] + t
                    nc.scalar.activation(
                        g_sb[:pcnt, gt * 128:gt * 128 + 128], pg[:pcnt, :],
                        AF.Copy)

            npc = sum(np_list)
            for c in range(NVCH):
                ps = spp.tile([128, 512], F32, tag="ps")
                pairs_c = []
                for mi in range(4):
                    for t in sched[mi][c]:
                        pairs_c.append((mi, t))
                half = (npc + 1) // 2
                wts = []
                for hb in range(2):
                    k0, k1 = hb * half, min((hb + 1) * half, npc)
                    wt = spw.tile([128, half, 512], BF16, tag="wsc")
                    nc.sync.dma_start(
                        wt[:, :k1 - k0, :],
                        ap["wsc"].rearrange("(k p) h -> p k h", p=128)
                        [:, c * npc + k0:c * npc + k1, :])
                    wts.append(wt)
                for k, (mi, t) in enumerate(pairs_c):
                    HW = MAPS[mi][1] ** 2
                    pcnt = min(128, HW - t * 128)
                    gt = g_off[mi] + t
                    nc.tensor.matmul(
                        out=ps[:],
                        lhsT=g_sb[:pcnt, gt * 128:gt * 128 + 128],
                        rhs=wts[k // half][:pcnt, k % half, :],
                        start=(k == 0), stop=(k == len(pairs_c) - 1))
                nc.scalar.activation(xx[:, c * 512:(c + 1) * 512], ps[:],
                                     AF.Relu)

        # ---------------- phase 2: graph conv layers ----------------
        lpool = ctx.enter_context(tc.tile_pool(name="h1c", bufs=1))
        h1c = lpool.tile([128, VP], F32, tag="h1c")
        lp = ctx.enter_context(tc.tile_pool(name="msg", bufs=3))
        lph = ctx.enter_context(tc.tile_pool(name="hst", bufs=2))
        lpv = ctx.enter_context(tc.tile_pool(name="vv", bufs=1))
        apb = ctx.enter_context(tc.tile_pool(name="apbuf", bufs=2))
        trp = ctx.enter_context(tc.tile_pool(name="trmsg", bufs=3))
        ohd = ctx.enter_context(tc.tile_pool(name="ohdma", bufs=2))
        pst = ctx.enter_context(tc.tile_pool(name="pstr", bufs=2, space="PSUM"))
        psh = ctx.enter_context(tc.tile_pool(name="psh", bufs=2, space="PSUM"))
        psx = ctx.enter_context(tc.tile_pool(name="psx", bufs=2, space="PSUM"))

        def emit_h1_rows(l, c0, nt4, h1_writes):
            """h1 rows for layer l, tiles [c0, c0+nt4) -> h1d2[l % 2]."""
            h1d = h1d2[l % 2]
            ph = psh.tile([128, 512], F32, tag="ph")
            if l == 0:
                vv = lpv.tile([3, 512], F32, tag="vt")
                nc.sync.dma_start(
                    vv[:, :nt4 * 128],
                    ap["vertsT"][:, c0 * 128:(c0 + nt4) * 128])
            for ti in range(nt4):
                t = c0 + ti
                sl = slice(ti * 128, (ti + 1) * 128)
                if l == 0:
                    nc.tensor.matmul(
                        out=ph[:, sl], lhsT=xx[:, t * 128:(t + 1) * 128],
                        rhs=g0m_t[:, 128:256], start=True, stop=False)
                    nc.tensor.matmul(
                        out=ph[:, sl], lhsT=vv[:, ti * 128:(ti + 1) * 128],
                        rhs=g0v_t[:, 128:256], start=False, stop=False)
                    nc.tensor.matmul(
                        out=ph[:, sl], lhsT=ones_t[:, 0:128],
                        rhs=erow_t[:, 128:256], start=False, stop=True)
                else:
                    nc.tensor.matmul(
                        out=ph[:, sl], lhsT=xx[:, t * 128:(t + 1) * 128],
                        rhs=w1_t[:, (l - 1) * 128:l * 128],
                        start=True, stop=True)
            hst = lph.tile([128, 512], BF16, tag="hst")
            nc.scalar.activation(hst[:, :nt4 * 128], ph[:, :nt4 * 128],
                                 AF.Copy)
            h1_writes.append(nc.sync.dma_start(
                h1d.rearrange("(n p) c -> p n c", p=128)[:, c0:c0 + nt4, :],
                hst[:, :nt4 * 128].rearrange("p (n c) -> p n c", c=128)))

        def emit_h1_cols(l, c):
            """h1 column chunk c (512 cols) for layer l -> h1c (f32)."""
            c0 = c * 512
            cw = 512
            ph = psh.tile([128, 512], F32, tag="ph")
            if l == 0:
                vv = lpv.tile([3, 512], F32, tag="vt")
                nc.sync.dma_start(vv[:, :cw], ap["vertsT"][:, c0:c0 + cw])
                nc.tensor.matmul(
                    out=ph[:], lhsT=g0m_t[:, 128:256],
                    rhs=xx[:, c0:c0 + cw], start=True, stop=False)
                nc.tensor.matmul(
                    out=ph[:], lhsT=g0v_t[:, 128:256],
                    rhs=vv[:, :cw], start=False, stop=False)
                nc.tensor.matmul(
                    out=ph[:], lhsT=erow_t[:, 128:256],
                    rhs=ones_t[:, :cw], start=False, stop=True)
            else:
                nc.tensor.matmul(
                    out=ph[:], lhsT=w1_t[:, (l - 1) * 128:l * 128],
                    rhs=xx[:, c0:c0 + cw], start=True, stop=True)
            nc.scalar.activation(h1c[:, c0:c0 + cw], ph[:], AF.Copy)

        CPW = WSZ // 512  # h1c chunks per window (7)

        def _layer(l, h1_writes, last_layer):
            """Scatter groups for layer l; h1 for layer l+1 is emitted inside
            (pipelined). Returns layer l+1's h1_writes list."""
            h1d = h1d2[l % 2]
            h1_writes_next = []
            trmsg_tiles = [[None] * len(calls[w]) for w in range(NW)]
            copy_flip = [0]

            def emit_call(w, k):
                s0, s1, _t0, _t1, _gs, _ge = calls[w][k]
                ns = s1 - s0
                buf = apb.tile([128, MAXCALL * 128], F32, tag="apbuf")
                nc.gpsimd.ap_gather(
                    out_ap=buf[:, :ns * 128],
                    in_ap=h1c[:, w * WSZ:(w + 1) * WSZ],
                    idxs_ap=apw_t[w][:, s0 * 8:s1 * 8],
                    channels=128, num_elems=WSZ, d=1, num_idxs=ns * 128)
                tr = trp.tile([128, MAXCALL, 128], BF16, tag="trmsg")
                trmsg_tiles[w][k] = (tr, s0)
                for j4 in range(0, ns, 4):
                    jn = min(4, ns - j4)
                    pt = pst.tile([128, 512], F32, tag="pt")
                    for j in range(jn):
                        src_sl = buf[:, (j4 + j) * 128:(j4 + j + 1) * 128]
                        id_sl = ident_t[:]
                        out_sl = pt[:, j * 128:(j + 1) * 128]
                        if TR_F32R:
                            src_sl = src_sl.bitcast(F32R)
                            id_sl = id_sl.bitcast(F32R)
                            out_sl = out_sl.bitcast(F32R)
                        nc.tensor.transpose(out_sl, src_sl, id_sl)
                    dst_sl = tr[:, j4:j4 + jn, :].rearrange("p s o -> p (s o)")
                    if copy_flip[0] % 2 == 0:
                        nc.vector.tensor_copy(dst_sl, pt[:, :jn * 128])
                    else:
                        nc.scalar.activation(dst_sl, pt[:, :jn * 128], AF.Copy)
                    copy_flip[0] += 1

            next_call = [0] * NW
            pending_cols = [] if last_layer else list(range(NVCH))

            gathered = {}  # g -> (msg, ohg) or None

            def emit_gather(g):
                t_lo = g * GT
                t_hi = min((g + 1) * GT, NT)
                s0, s1 = sub_off[t_lo], sub_off[t_hi]
                ng = s1 - s0
                if ng == 0:
                    gathered[g] = None
                    return
                msg = lp.tile([128, MAXSUB_G, 128], BF16, tag="msg")
                gi = nc.gpsimd.dma_gather(
                    out_ap=msg[:, :ng, :],
                    in_ap=h1d[:],
                    idxs_ap=srcw_t[:, s0 * 8:s1 * 8],
                    num_idxs=ng * 128,
                    num_idxs_reg=ng * 128,
                    elem_size=HID,
                    single_packet=False,
                )
                for wi in h1_writes:
                    tile.add_dep_helper(gi.ins, wi.ins,
                                        reason="h1 RAW: gather after write")
                ohg = ohd.tile([128, MAXSUB_G, 128], FP8, tag="ohg")
                nc.vector.tensor_tensor(
                    out=ohg[:, :ng, :],
                    in0=dstloc_t[:, s0:s1, :].to_broadcast([128, ng, 128]),
                    in1=iota_t[:].to_broadcast([128, ng, 128]),
                    op=mybir.AluOpType.is_equal)
                gathered[g] = (msg, ohg)

            PREFETCH = 2
            for g in range(min(PREFETCH, NGRP)):
                emit_gather(g)

            for g in range(NGRP):
                t_lo = g * GT
                t_hi = min((g + 1) * GT, NT)
                for w in range(NW):
                    while (next_call[w] < len(calls[w])
                           and calls[w][next_call[w]][4] <= g):
                        emit_call(w, next_call[w])
                        next_call[w] += 1
                if g + PREFETCH < NGRP:
                    emit_gather(g + PREFETCH)

                W_ = (t_hi - t_lo) * 128
                px = psx.tile([128, GT * 128], F32, tag="px")

                s0, s1 = sub_off[t_lo], sub_off[t_hi]
                ng = s1 - s0
                if ng > 0:
                    msg, ohg = gathered.pop(g)

                # collect accumulating matmuls; psum-bank-sized segments get
                # their own start/stop
                mms = []  # entries: (seg_id, kwargs)
                if l == 0:
                    vv2 = lpv.tile([3, GT * 128], F32, tag="vt2")
                    nc.sync.dma_start(
                        vv2[:, :W_], ap["vertsT"][:, t_lo * 128:t_hi * 128])
                    for seg in range(0, W_, 512):
                        sw = min(512, W_ - seg)
                        c0 = t_lo * 128 + seg
                        mms.append((seg // 512,
                                    dict(out=px[:, seg:seg + sw],
                                         lhsT=g0m_t[:, 0:128],
                                         rhs=xx[:, c0:c0 + sw])))
                        mms.append((seg // 512,
                                    dict(out=px[:, seg:seg + sw],
                                         lhsT=g0v_t[:, 0:128],
                                         rhs=vv2[:, seg:seg + sw])))
                        mms.append((seg // 512,
                                    dict(out=px[:, seg:seg + sw],
                                         lhsT=erow_t[:, 0:128],
                                         rhs=ones_t[:, :sw])))
                else:
                    for seg in range(0, W_, 512):
                        sw = min(512, W_ - seg)
                        c0 = t_lo * 128 + seg
                        mms.append((seg // 512,
                                    dict(out=px[:, seg:seg + sw],
                                         lhsT=w0_t[:, (l - 1) * 128:l * 128],
                                         rhs=xx[:, c0:c0 + sw])))
                for ti in range(t_hi - t_lo):
                    t = t_lo + ti
                    osl = slice(ti * 128, (ti + 1) * 128)
                    for j in range(sub_off[t] - s0, sub_off[t + 1] - s0):
                        mms.append((ti * 128 // 512,
                                    dict(out=px[:, osl], lhsT=msg[:, j, :],
                                         rhs=ohg[:, j, :])))
                    for w in range(NW):
                        na = n_ap[t][w]
                        if na == 0:
                            continue
                        kk = next(
                            i for i, c in enumerate(calls[w])
                            if c[2] <= t < c[3])
                        tr, trs0 = trmsg_tiles[w][kk]
                        for j in range(na):
                            s_loc = ap_off[t][w] - trs0 + j
                            s_ap = ap_base[w] - S_dma + ap_off[t][w] + j
                            mms.append((ti * 128 // 512,
                                        dict(out=px[:, osl],
                                             lhsT=tr[:, s_loc, :],
                                             rhs=oh_ap[:, s_ap, :])))
                first_of = {}
                last_of = {}
                for i, (sg, _kw) in enumerate(mms):
                    first_of.setdefault(sg, i)
                    last_of[sg] = i
                for i, (sg, kw) in enumerate(mms):
                    nc.tensor.matmul(start=(first_of[sg] == i),
                                     stop=(last_of[sg] == i),
                                     skip_group_check=True, **kw)
                nc.scalar.activation(xx[:, t_lo * 128:t_hi * 128], px[:, :W_],
                                     AF.Relu)

                # ---- pipelined layer-(l+1) h1 production ----
                if not last_layer:
                    for c0 in range(t_lo, t_hi, 4):
                        emit_h1_rows(l + 1, c0, min(4, t_hi - c0),
                                     h1_writes_next)
                    still = []
                    for c in pending_cols:
                        w = c // CPW
                        src_g = (c * 4 + 3) // GT
                        if next_call[w] == len(calls[w]) and src_g <= g:
                            emit_h1_cols(l + 1, c)
                        else:
                            still.append(c)
                    pending_cols = still
            assert not pending_cols
            return h1_writes_next

        for _rep in range(repeat):
            h1w = []
            for c0 in range(0, NT, 4):
                emit_h1_rows(0, c0, min(4, NT - c0), h1w)
            for c in range(NVCH):
                emit_h1_cols(0, c)
            for l in range(nlayers):
                h1w = _layer(l, h1w, l == nlayers - 1)

        # ---------------- output ----------------
        OB = 12
        for t0 in range(0, NT, OB):
            tb = min(OB, NT - t0)
            ost = lph.tile([128, OB * 3], F32, tag="ost")
            for ti in range(tb):
                t = t0 + ti
                po = psh.tile([128, 512], F32, tag="ph")
                nc.tensor.matmul(out=po[:, :3],
                                 lhsT=xx[:, t * 128:(t + 1) * 128],
                                 rhs=offw_t[:], start=True, stop=True)
                nc.scalar.activation(ost[:, ti * 3:(ti + 1) * 3], po[:, :3],
                                     AF.Copy)
            nc.sync.dma_start(
                out.rearrange("(n p) c -> p n c", p=128)[:, t0:t0 + tb, :],
                ost[:, :tb * 3].rearrange("p (n c) -> p n c", c=3))

    nc.compile()
    return nc


_CACHE = {}


def kernel(**inputs) -> np.ndarray:
    cfg, per_core, post = _prep(inputs)
    key = (cfg["npair"], cfg["S_tot"], cfg["S_dma"],
           tuple(cfg["np_list"]), str(cfg["calls"]))
    if key not in _CACHE:
        _CACHE[key] = _build(cfg, per_core[0])
    nc = _CACHE[key]
    res = run_bass_kernel_spmd(nc, per_core, list(range(B)))
    outs = np.empty((B, V, 3), np.float32)
    for m in range(B):
        rows = res.results[m]["out"]
        outs[m] = rows[post["slot_of"][m][np.arange(V)]]
    return outs.reshape(B * V, 3)


if __name__ == "__main__":
    pass
